# revision 15
# baseline (speedup 1.0000x reference)
"""CBAM attention module (channel gate + spatial softmax attention) on 8 TRN2
NeuronCores, data-parallel over the batch dimension.

Reference computation (per sample b):
    m  = mean_n x[c, n];  mx = max_n x[c, n]
    gate = sigmoid(w2 @ (relu(w1 @ m) + relu(w1 @ mx)))          # (C,)
    x1 = gate[:, None] * x
    s  = sw0 * max_c x1 + sw1 * mean_c x1                        # (N,)
    s  = relu(A * s + Bconst)        # BatchNorm1d(1) eval, folded on host
    att = softmax_n(s)
    out = att[None, :] * x1

v2 layout: x is shipped to the device in bf16 (host cast; stats/products
keep f32 accumulation so the rel-err stays ~1e-3, far under the 2e-2 gate)
and the output is written bf16 and upcast on the host.  That halves all
four HBM sweeps: 3 reads + 1 write of 32 MiB bf16 per sample = 256 MiB per
core (2 samples), vs 512 MiB for the all-f32 pipeline.

Per-core passes (2 samples each):
    pass 1: stream x; VectorE per-channel max, ScalarE activation-accum sum
            -> tiny MLP on TensorE -> gate (bf16)
    pass 2: stream x; fused DVE scalar_tensor_tensor (x*gate) running-max
            over the 8 c-chunks; TensorE gate-stationary matvec accumulates
            the c-sum in PSUM row-pieces; TensorE transposes 128x128 blocks
            of the max so VectorE can reduce over c.  Softmax over n in the
            transposed layout.
    pass 3: stream x; att replicated across partitions by a TensorE
            ones-outer-product; one fused DVE op computes x*gate*att.

Engine budget at the 0.84 ms DMA roofline: vector ~0.36 ms, scalar
~0.31 ms, PE ~0.34 ms, sync dispatch ~0.29 ms - DMA-bound everywhere.
"""

import numpy as np
import ml_dtypes

B, C, N, RATIO = 16, 1024, 16384, 8
H = C // RATIO  # 128
BN_EPS = 1e-5
N_CORES = 8
BC = B // N_CORES  # samples per core

_cached = {}


def _build_nc(NT=4096, NT2=4096, BC=BC, C=C, N=N, H=H):
    import concourse.bacc as bacc
    import concourse.mybir as mybir
    import concourse.tile as tile
    from concourse import masks
    from contextlib import ExitStack

    f32 = mybir.dt.float32
    bf16 = mybir.dt.bfloat16
    fp8 = mybir.dt.float8e4
    AF = mybir.ActivationFunctionType
    X = mybir.AxisListType.X
    OP = mybir.AluOpType

    K = C // 128          # c-chunks
    NJ = N // NT          # n-tiles per sample (passes 1/3)
    NJ2 = N // NT2        # pass-2 tiles per sample
    MV = NT2 // 512       # matvec row-pieces per pass-2 tile (PSUM banks)
    NB = N // 128         # transpose-layout columns
    BPT2 = NT2 // 128     # 128-blocks per pass-2 tile
    assert NB <= 128

    nc = bacc.Bacc("TRN2", target_bir_lowering=False, debug=False,
                   num_devices=N_CORES)

    x = nc.dram_tensor("x", (BC, C, N), bf16, kind="ExternalInput").ap()
    w1t = nc.dram_tensor("w1t", (C, H), f32, kind="ExternalInput").ap()
    w2t = nc.dram_tensor("w2t", (H, C), f32, kind="ExternalInput").ap()
    # params = [sw0, sw1/C, A, Bconst]
    params = nc.dram_tensor("params", (1, 4), f32, kind="ExternalInput").ap()
    out = nc.dram_tensor("out", (BC, C, N), bf16, kind="ExternalOutput").ap()

    att_dram = nc.dram_tensor("att_scratch", (BC, N), bf16, kind="Internal").ap()
    cm_dram = nc.dram_tensor("cm_scratch", (BC, N), f32, kind="Internal").ap()

    with tile.TileContext(nc) as tc, ExitStack() as ctx:
        consts = ctx.enter_context(tc.tile_pool(name="consts", bufs=1))
        big = ctx.enter_context(tc.tile_pool(name="big", bufs=2))
        small = ctx.enter_context(tc.tile_pool(name="small", bufs=3))
        psum = ctx.enter_context(tc.tile_pool(name="psum", bufs=2, space="PSUM"))

        # ---- constants ----
        identity = consts.tile([128, 128], f32)
        masks.make_identity(nc, identity)
        identity_b = consts.tile([128, 128], bf16)
        masks.make_identity(nc, identity_b)
        ones_row = consts.tile([1, 128], f32)
        nc.vector.memset(ones_row, 1.0)
        ones_row_b = consts.tile([1, 128], bf16)
        nc.vector.memset(ones_row_b, 1.0)
        params_sb = consts.tile([128, 4], f32)
        nc.sync.dma_start(out=params_sb, in_=params.to_broadcast((128, 4)))
        w1t_sb = consts.tile([128, K, H], f32)
        nc.sync.dma_start(out=w1t_sb, in_=w1t.rearrange("(k p) h -> p k h", p=128))
        w2t_sb = consts.tile([H, C], f32)
        nc.sync.dma_start(out=w2t_sb, in_=w2t)

        # ---- persistent stats ----
        mx_cols = consts.tile([128, BC, K, NJ], f32)
        sum_cols = consts.tile([128, BC, K, NJ], f32)
        stats = consts.tile([128, K, BC, 2], f32)   # per (k, b): [sum, max]
        gate_b = consts.tile([128, K, BC], bf16)
        gate_f = consts.tile([128, K, BC], f32)
        cx_t = consts.tile([128, BC, NB], f32)
        cmrows = consts.tile([NB, BC, 128], f32)

        xrs = [x[b].rearrange("(k p) n -> p k n", p=128) for b in range(BC)]
        outrs = [out[b].rearrange("(k p) n -> p k n", p=128) for b in range(BC)]

        def load_chunk(b, k, j, nt, tag, bufs=4, eng=None):
            t = big.tile([128, nt], bf16, tag=tag, bufs=bufs, name=tag)
            (eng or nc.sync).dma_start(
                out=t, in_=xrs[b][:, k, j * nt:(j + 1) * nt])
            return t

        # ---------------- pass 1: per-channel sum & max over n -------------
        def p1_iter(b, j):
            for k in range(K):
                xk = load_chunk(b, k, j, NT, tag="xin1", bufs=3)
                nc.vector.reduce_max(out=mx_cols[:, b, k, j:j + 1],
                                     in_=xk, axis=X)
                dummy = big.tile([128, NT], fp8, tag="dummy")
                nc.scalar.activation(out=dummy, in_=xk, func=AF.Copy,
                                     accum_out=sum_cols[:, b, k, j:j + 1])

        # ---------------- MLP -> gate (per sample) -------------------------
        def mlp(b):
            nc.vector.reduce_sum(out=stats[:, :, b, 0:1],
                                 in_=sum_cols[:, b, :, :], axis=X)
            nc.vector.reduce_max(out=stats[:, :, b, 1:2],
                                 in_=mx_cols[:, b, :, :], axis=X)
            h_psum = psum.tile([H, 2], f32, tag="tp", name="h_psum")
            for k in range(K):
                nc.tensor.matmul(h_psum, lhsT=w1t_sb[:, k, :],
                                 rhs=stats[:, k, b, :],
                                 start=(k == 0), stop=(k == K - 1))
            hr = small.tile([H, 2], f32, tag="hr")
            nc.scalar.activation(out=hr[:, 0:1], in_=h_psum[:, 0:1],
                                 func=AF.Relu, scale=1.0 / N)
            nc.scalar.activation(out=hr[:, 1:2], in_=h_psum[:, 1:2],
                                 func=AF.Relu, scale=1.0)
            hsum = small.tile([H, 1], f32, tag="hsum")
            nc.vector.tensor_add(out=hsum, in0=hr[:, 0:1], in1=hr[:, 1:2])
            for k in range(K):
                g_psum = psum.tile([128, 1], f32, tag="tp", name="g_psum")
                nc.tensor.matmul(g_psum, lhsT=w2t_sb[:, k * 128:(k + 1) * 128],
                                 rhs=hsum, start=True, stop=True)
                nc.scalar.activation(out=gate_b[:, k, b:b + 1], in_=g_psum,
                                     func=AF.Sigmoid)
                nc.scalar.activation(out=gate_f[:, k, b:b + 1], in_=g_psum,
                                     func=AF.Sigmoid)

        # ---------------- pass 2: x1 stats over c --------------------------
        def p2_iter(b, j):
            # c-sum: gate (stationary, bf16) @ x rows -> [1, 512] row-pieces
            # accumulating over k; two pieces share a PSUM bank (rows 0/1).
            mv_banks = [psum.tile([128, 512], f32, tag=f"mv{q}", bufs=1,
                                  name=f"mv{q}")
                        for q in range(MV // 2)]
            # fused (x * gate) running-max, ping-pong between two buffers
            tmaxes = [big.tile([128, NT2], bf16, tag=f"tmax{i}", bufs=1,
                               name=f"tmax{i}")
                      for i in range(2)]
            for k in range(K):
                xk = load_chunk(b, k, j, NT2, tag="xin2", bufs=4,
                                eng=nc.scalar)
                for p in range(MV):
                    row = (p % 2) * 64
                    nc.tensor.matmul(mv_banks[p // 2][row:row + 1, :],
                                     lhsT=gate_b[:, k, b:b + 1],
                                     rhs=xk[:, p * 512:(p + 1) * 512],
                                     start=(k == 0), stop=(k == K - 1))
                if k == 0:
                    nc.vector.tensor_scalar(out=tmaxes[0], in0=xk,
                                            scalar1=gate_f[:, k, b:b + 1],
                                            scalar2=None, op0=OP.mult)
                else:
                    nc.vector.scalar_tensor_tensor(
                        out=tmaxes[k % 2], in0=xk,
                        scalar=gate_f[:, k, b:b + 1],
                        in1=tmaxes[1 - (k % 2)], op0=OP.mult, op1=OP.max)
            tm = tmaxes[(K - 1) % 2]
            # max over c: transpose 128x128 blocks, 4 blocks per PSUM bank,
            # then one 3D-view reduce per bank (out [128, 4])
            for bk in range(BPT2 // 4):
                tpb = psum.tile([128, 4, 128], bf16, tag="tp")
                for q in range(4):
                    blk = bk * 4 + q
                    nc.tensor.transpose(tpb[:, q, :],
                                        tm[:, blk * 128:(blk + 1) * 128],
                                        identity_b)
                col = j * BPT2 + bk * 4
                nc.vector.reduce_max(out=cx_t[:, b, col:col + 4], in_=tpb,
                                     axis=X)
            # stage c-sum row-pieces through DRAM for the softmax transpose
            for p in range(MV):
                cm_stage = small.tile([1, 512], f32, tag="cmstage",
                                      name="cm_stage")
                nc.scalar.copy(out=cm_stage,
                               in_=mv_banks[p // 2][(p % 2) * 64:
                                                    (p % 2) * 64 + 1, :])
                n0 = j * NT2 + p * 512
                nc.sync.dma_start(out=cm_dram[b:b + 1, n0:n0 + 512],
                                  in_=cm_stage)

        # ---------------- softmax over n (transpose layout) ----------------
        def softmax(b):
            nc.sync.dma_start(
                out=cmrows[:, b, :],
                in_=cm_dram[b].rearrange("(jj p) -> jj p", p=128))
            cmt_psum = psum.tile([128, NB], f32, tag="tp", name="cmt_psum")
            nc.tensor.transpose(cmt_psum, cmrows[:, b, :],
                                identity[0:NB, 0:NB])
            s_t = small.tile([128, NB], f32, tag="st")
            # s = sw0 * cx + (sw1/C) * cm_sum
            nc.vector.tensor_scalar(out=s_t, in0=cmt_psum,
                                    scalar1=params_sb[:, 1:2], scalar2=None,
                                    op0=OP.mult)
            tmp_t = small.tile([128, NB], f32, tag="st2")
            nc.vector.tensor_scalar(out=tmp_t, in0=cx_t[:, b, :],
                                    scalar1=params_sb[:, 0:1], scalar2=None,
                                    op0=OP.mult)
            nc.vector.tensor_add(out=s_t, in0=s_t, in1=tmp_t)
            # BN (affine, host-folded) + relu
            nc.scalar.activation(out=s_t, in_=s_t, func=AF.Relu,
                                 scale=params_sb[:, 2:3],
                                 bias=params_sb[:, 3:4])

            # global max/sum over all partitions via PE transpose + ones
            # broadcast
            def preduce(col, op, nm):
                row_ps = psum.tile([1, 128], f32, tag="tp", name=nm + "_r")
                nc.tensor.transpose(row_ps, col, identity)
                scl = small.tile([1, 1], f32, tag=nm + "s", name=nm + "_s")
                nc.vector.tensor_reduce(out=scl, in_=row_ps, axis=X, op=op)
                rep_ps = psum.tile([128, 1], f32, tag="tp", name=nm + "_b")
                nc.tensor.matmul(rep_ps, lhsT=ones_row, rhs=scl,
                                 start=True, stop=True)
                rep = small.tile([128, 1], f32, tag=nm, name=nm)
                nc.scalar.copy(out=rep, in_=rep_ps)
                return rep

            colmax = small.tile([128, 1], f32, tag="cmax")
            nc.vector.reduce_max(out=colmax, in_=s_t, axis=X)
            gmax = preduce(colmax, OP.max, "gmax")
            ngmax = small.tile([128, 1], f32, tag="ngmax")
            nc.vector.tensor_scalar(out=ngmax, in0=gmax, scalar1=-1.0,
                                    scalar2=None, op0=OP.mult)
            e_t = small.tile([128, NB], f32, tag="et")
            sume = small.tile([128, 1], f32, tag="sume")
            nc.scalar.activation(out=e_t, in_=s_t, func=AF.Exp, bias=ngmax,
                                 scale=1.0, accum_out=sume)
            gsum = preduce(sume, OP.add, "gsum")
            rinv = small.tile([128, 1], f32, tag="rinv")
            nc.vector.reciprocal(out=rinv, in_=gsum)
            att_t = small.tile([128, NB], f32, tag="attt")
            nc.vector.tensor_scalar(out=att_t, in0=e_t, scalar1=rinv,
                                    scalar2=None, op0=OP.mult)
            # transpose-layout -> row-major (jj on partitions), cast to bf16
            attt_psum = psum.tile([NB, 128], f32, tag="tp", name="attt_psum")
            nc.tensor.transpose(attt_psum, att_t, identity)
            att_rows = small.tile([NB, 128], bf16, tag="attrows")
            nc.scalar.copy(out=att_rows, in_=attt_psum)
            nc.sync.dma_start(
                out=att_dram[b].rearrange("(jj p) -> jj p", p=128),
                in_=att_rows)

        # ---------------- pass 3: out = att * gate * x ---------------------
        def p3_iter(b, j):
            att_piece = small.tile([1, NT], bf16, tag="attp", bufs=2)
            nc.sync.dma_start(out=att_piece,
                              in_=att_dram[b:b + 1, j * NT:(j + 1) * NT])
            attr = big.tile([128, NT], bf16, tag="attr", bufs=2)
            for p in range(NT // 512):
                bc_psum = psum.tile([128, 512], f32, tag="attrp",
                                    name="bc_psum")
                nc.tensor.matmul(bc_psum, lhsT=ones_row_b,
                                 rhs=att_piece[:, p * 512:(p + 1) * 512],
                                 start=True, stop=True)
                nc.scalar.copy(out=attr[:, p * 512:(p + 1) * 512],
                               in_=bc_psum)
            for k in range(K):
                xk = load_chunk(b, k, j, NT, tag="xin3", bufs=3)
                yout = big.tile([128, NT], bf16, tag="yout", bufs=3)
                nc.vector.scalar_tensor_tensor(
                    out=yout, in0=xk, scalar=gate_f[:, k, b:b + 1],
                    in1=attr, op0=OP.mult, op1=OP.mult)
                nc.sync.dma_start(out=outrs[b][:, k, j * NT:(j + 1) * NT],
                                  in_=yout)

        # ---------------- emission schedule (software pipeline) ------------
        # P2 is vector-heavy but DMA-light; interleave it with the DMA-heavy
        # P1/P3 streams of the other sample so neither resource idles.
        if BC == 2 and NJ2 == NJ == 4:
            for j in range(NJ):
                p1_iter(0, j)
            mlp(0)
            # front-load P1(b1) so mlp(b1) lands early and P2(b1) can
            # interleave with the tail of P2(b0); keeps VectorE uniformly
            # busy instead of piling both P2 streams into one phase
            p1_iter(1, 0)
            p1_iter(1, 1)
            p2_iter(0, 0)
            p1_iter(1, 2)
            p1_iter(1, 3)
            mlp(1)
            p2_iter(0, 1)
            p2_iter(1, 0)
            p2_iter(0, 2)
            p2_iter(1, 1)
            p2_iter(0, 3)
            softmax(0)
            p2_iter(1, 2)
            p3_iter(0, 0)
            p2_iter(1, 3)
            p3_iter(0, 1)
            softmax(1)
            p3_iter(0, 2)
            p3_iter(1, 0)
            p3_iter(0, 3)
            p3_iter(1, 1)
            p3_iter(1, 2)
            p3_iter(1, 3)
        else:
            for b in range(BC):
                for j in range(NJ):
                    p1_iter(b, j)
            for b in range(BC):
                mlp(b)
                for j in range(NJ2):
                    p2_iter(b, j)
                softmax(b)
                for j in range(NJ):
                    p3_iter(b, j)

    nc.compile()
    return nc


def _get_nc(NT=4096):
    key = ("nc", NT)
    if key not in _cached:
        _cached[key] = _build_nc(NT)
    return _cached[key]


def _host_params(sw, gamma, beta, running_mean, running_var):
    A = float(gamma[0]) / np.sqrt(float(running_var[0]) + BN_EPS)
    Bconst = float(beta[0]) - float(running_mean[0]) * A
    return np.array([[float(sw[0]), float(sw[1]) / C, A, Bconst]],
                    dtype=np.float32)


def _make_in_maps(x, w1, w2, sw, gamma, beta, running_mean, running_var):
    xb = np.asarray(x, dtype=np.float32).astype(ml_dtypes.bfloat16)
    w1t = np.ascontiguousarray(np.asarray(w1, dtype=np.float32).T)
    w2t = np.ascontiguousarray(np.asarray(w2, dtype=np.float32).T)
    params = _host_params(np.asarray(sw), np.asarray(gamma), np.asarray(beta),
                          np.asarray(running_mean), np.asarray(running_var))
    in_maps = []
    for core in range(N_CORES):
        xs = np.ascontiguousarray(xb[core * BC:(core + 1) * BC])
        in_maps.append({"x": xs, "w1t": w1t, "w2t": w2t, "params": params})
    return in_maps


def run_sharded(inputs, trace=False, NT=4096):
    """Run on all 8 cores; returns (out_full, BassKernelResults)."""
    from concourse.bass_utils import run_bass_kernel_spmd

    nc = _get_nc(NT)
    in_maps = _make_in_maps(**inputs)
    res = run_bass_kernel_spmd(nc, in_maps, core_ids=list(range(N_CORES)),
                               trace=trace)
    out = np.concatenate(
        [np.asarray(r["out"]).astype(np.float32) for r in res.results], axis=0)
    return out, res


def kernel(**inputs) -> np.ndarray:
    out, _ = run_sharded(inputs, trace=False)
    return out
